# revision 71
# baseline (speedup 1.0000x reference)
# Trainium2 Bass kernel for GQA attention block (B=8, T=512, C=2048, 16 q heads,
# 4 kv heads, head_dim=128, RoPE, causal mask, output projection).
#
# Strategy: data parallel across the 8 NeuronCores — core i handles batch
# element i with the full weight set (no collectives). Per core everything is
# computed in a transposed layout:
#   qT/kT [d, t]  = W[:, d-tile].T-contract  (lhsT = weight tile, rhs = xT)
#   scoresT [s,t] = kT-slice.T @ qT
#   outT [d, t]   = v-slice.T @ (mask*exp(scoresT))  (unnormalized)
#   y [t, e]      = outT-slice.T @ wo-tile   (normalized outT)
# The softmax normalizer folds the 4 s-tiles of exp(scoresT) range-wise on
# the DVE, then one N=512 ones-matmul broadcasts the partition-sum to all
# 128 partitions (vs 4 matmuls of the unfolded tiles).
# RoPE rotate-half is done with partition-shifted PSUM reads (ACT copies) +
# 2 muls + 1 add; sin rows 0:64 are pre-negated on the host.
#
# Performance structure (~189us/core at the warm 2.4GHz PE clock, ~80% MFU):
# - all inputs pre-tiled partition-major on the host so every DMA reads
#   per-partition-contiguous ranges (descriptor-efficient, ~400GB/s streams)
# - phase A fuses the V and K projections ct-major (8 dense matmuls per ct)
#   against the arriving xT/weight streams; the last group runs K j-major so
#   the kp PSUM banks release early for the first q-head
# - dummy/zero-contribution matmuls bridge the DMA ramp so the PE HAM clock
#   gate lifts the 1.2GHz idle throttle before the real stream begins
# - phase B pipelines 16 q-heads (proj -> rope -> scores/exp -> AV/norm)
# - phase C accumulates each [128,512] y-tile fully, then copies+DMAs it
#   immediately (fp16 output, cast to f32 on host); the final tile is two
#   256-col chains so only a 64KB copy+DMA trails the last matmul
# Matmul operands are fp16 (full PE rate, half the HBM bytes, DVE 2x modes;
# fp32 PSUM accumulation). When the runtime mask is exactly causal, the
# scores/AV matmuls restrict their moving dim to the visible t-range and only
# the diagonal 128x128 block gets mask-multiplied; any other mask falls back
# to a general masked build.

import os
import sys

import numpy as np

for _p in (
    "/root/.axon_site",
    "/root/.axon_site/_ro/trn_rl_repo",
    "/root/.axon_site/_ro/pypackages",
    "/opt/trn_rl_repo",
):
    if os.path.isdir(_p) and _p not in sys.path:
        sys.path.append(_p)

import concourse.bass as bass  # noqa: E402
import concourse.mybir as mybir  # noqa: E402
import concourse.tile as tile  # noqa: E402
from concourse import bacc  # noqa: E402
from concourse.bass_utils import run_bass_kernel_spmd  # noqa: E402

F32 = mybir.dt.float32
F32R = mybir.dt.float32r
F16 = mybir.dt.float16
AF = mybir.ActivationFunctionType

B, T, C = 8, 512, 2048
HD, NH, NKV = 128, 16, 4
CT = C // 128  # 16 contraction tiles
TT = T // 128  # 4 t/s tiles
EG = C // 512  # 4 output column groups
REP = NH // NKV
SCALE = float(HD) ** -0.5
N_CORES = 8

# Matmul operand dtype. fp16 (default): full PE rate, half the DMA bytes,
# DVE 2x modes; fp32r: full-rate reduced-precision fp32; fp32: exact, 4x
# slower on the PE.
_DT_ENV = os.environ.get("ATTN_DTYPE", "fp16")
MM_DT = {"fp16": F16, "fp32r": F32R, "fp32": F32}[_DT_ENV]
MM_NP = {"fp16": np.float16, "fp32r": np.float32, "fp32": np.float32}[_DT_ENV]


def _s(i, n):
    return slice(i * n, (i + 1) * n)




def _emit(tc, xT, wq, wk, wv, wo, cosT, sinT, maskT, y, causal):
    nc = tc.nc
    mm = nc.tensor.matmul

    with (
        tc.tile_pool(name="consts", bufs=1) as consts,
        tc.tile_pool(name="streams", bufs=2) as streams,
        tc.tile_pool(name="work", bufs=2) as work,
        tc.tile_pool(name="ps", bufs=1, space="PSUM") as ps,
    ):
        cosT_sb = consts.tile([HD, T], MM_DT)
        sinT_sb = consts.tile([HD, T], MM_DT)
        maskT_sb = consts.tile([128, TT, T], MM_DT)
        ones_sb = consts.tile([128, 128], MM_DT)
        xT_sb = consts.tile([128, CT, T], MM_DT)
        kT_sb = consts.tile([HD, NKV, T], MM_DT)
        v_sb = consts.tile([128, TT, 4 * HD], MM_DT)
        # two tiles so phase C's early wo-matmuls (heads 0..7) don't pick up
        # a scheduler dependency on the last head's write
        aouts = [consts.tile([HD, 4, T], MM_DT, name=f"aout{g}") for g in range(4)]

        def aout_slice(h, ts=slice(None)):
            return aouts[h // 4][:, h % 4, ts]

        # ---- PE warm-up: the HAM clock gate needs ~3.4us of sustained PE
        # activity before it lifts the idle throttle; dummy matmuls on a
        # zeroed scratch tile keep the PE busy through the DMA ramp. The
        # scratch PSUM shares the "big" slot, released before vp's first use.
        warm_sb = consts.tile([128, 128], MM_DT)
        gate_sb = consts.tile([1, 2], MM_DT)
        nc.gpsimd.memset(warm_sb[:], 0.0)
        nc.gpsimd.memset(ones_sb[:], 1.0)
        warm_ps = ps.tile([128, 128], F32, tag="big", name="warm")
        for _w in range(68):
            mm(warm_ps[:], warm_sb[:], warm_sb[:], start=True, stop=True)

        # wq head-slice prefetch (sync ring, off the ACT queue), depth = bufs
        wqh_tiles = {}

        def wqh_dma(h, eng=None):
            if h >= NH:
                return
            wqh = streams.tile([128, CT, HD], MM_DT, tag="wqh", bufs=4, name=f"wqh{h}")
            (eng or nc.sync).dma_start(wqh[:], wq[:, h])
            wqh_tiles[h] = wqh

        # ---- rope helper: psrc (PSUM, [d, t] f32) -> out_slice (SBUF) ----
        # rotate-half via partition-shifted PSUM reads; sinT_sb rows 0:64 are
        # pre-negated on the host, so no rotation matmul is needed.
        def rope(psrc, out_slice, tag):
            qrot = work.tile([HD, T], MM_DT, tag="trot", name=f"qrot_{tag}")
            nc.scalar.copy(qrot[0:64, :], psrc[64:128, :])
            nc.scalar.copy(qrot[64:128, :], psrc[0:64, :])
            tcos = work.tile([HD, T], MM_DT, tag="tcos", name=f"tcos_{tag}")
            nc.vector.tensor_mul(tcos[:], psrc, cosT_sb[:])
            nc.vector.tensor_mul(qrot[:], qrot[:], sinT_sb[:])
            nc.vector.tensor_add(out_slice, tcos[:], qrot[:])

        # first xT tiles via the scalar HWDGE ring (low latency for the very
        # first matmuls); the rest + constants via SWDGE, keeping the sync
        # ring free for the weight streams and the ACT queue for compute.
        # The ct=0 tile is split so the first matmul's stationary operand
        # (t-cols 0:128) lands as early as possible.
        nc.scalar.dma_start(xT_sb[:, 0, 0:64], xT[:, 0, 0:64])
        nc.scalar.dma_start(xT_sb[:, 0, 64:128], xT[:, 0, 64:128])
        nc.scalar.dma_start(xT_sb[:, 0, 128:T], xT[:, 0, 128:T])

        # ---- phase A: fused v + k projections, ct-major so the 8 matmuls
        # per ct keep the PE dense while the xT/weight streams arrive;
        # rope-k runs inside the phase-B pipeline ----
        vp = ps.tile([128, TT, 4 * HD], F32, tag="big", bufs=1)
        _kp_tags = (("qp", 2), ("qp", 2), ("av", 1), ("lsum", 1))
        kps = [
            ps.tile([HD, T], F32, tag=t, bufs=bf, name=f"kp{j}")
            for j, (t, bf) in enumerate(_kp_tags)
        ]
        _groups = [(0, 1), (1, 1), (2, 2), (4, 2), (6, 2), (8, 2), (10, 2), (12, 2), (14, 2)]
        _fillers = {1: 6, 2: 8, 3: 8, 4: 16, 5: 16, 6: 6, 7: 4}
        _last = len(_groups) - 1
        for nch, (c0, ncs) in enumerate(_groups):
            vwt = streams.tile(
                [128, 4, 4 * HD], MM_DT, tag="wkv4", bufs=8, name=f"vwt{nch}"
            )
            kwt = streams.tile(
                [128, 4, 4 * HD], MM_DT, tag="wkv4", bufs=8, name=f"kwt{nch}"
            )
            # scalar ring in consumption order: this group's xT tiles, then
            # its k weights (sync carries the v weights in parallel)
            for ct in range(max(c0, 1), min(c0 + ncs, 10)):
                nc.scalar.dma_start(xT_sb[:, ct, :], xT[:, ct, :])
            if nch == 0:
                # split the first weight chunk so the first real matmul only
                # waits on 64KB of DMA
                nc.sync.dma_start(vwt[:, 0, 0:256], wv[:, 0, 0:256])
                nc.sync.dma_start(vwt[:, 0, 256:512], wv[:, 0, 256:512])
            else:
                nc.sync.dma_start(vwt[:, :ncs, :], wv[:, c0 : c0 + ncs, :])
            # late k-weight groups ride the (quieter) scalar ring so the
            # sync ring's v-weights for groups 4-6 arrive sooner
            (nc.scalar if nch >= 4 else nc.sync).dma_start(
                kwt[:, :ncs, :], wk[:, c0 : c0 + ncs, :]
            )
            if nch == _last:
                # last group j-major for K, so each kp chain finishes several
                # matmuls early and the k-ropes (which gate the first q-head's
                # PSUM slot) overlap the remaining phase-A matmuls
                for j in range(NKV):
                    for ci in range(ncs):
                        mm(
                            kps[j][:],
                            kwt[:, ci, _s(j, HD)],
                            xT_sb[:, c0 + ci, :],
                            start=False,
                            stop=(ci == ncs - 1),
                        )
                for ci in range(ncs):
                    for i in range(TT):
                        mm(
                            vp[:, i, :],
                            xT_sb[:, c0 + ci, _s(i, 128)],
                            vwt[:, ci, :],
                            start=False,
                            stop=(ci == ncs - 1),
                        )
            else:
                for ci in range(ncs):
                    ct = c0 + ci
                    for i in range(TT):
                        if ct == 0 and i == 0:
                            # start=True clears the whole bank; the second
                            # half overwrites its (untouched) element range
                            mm(vp[:, 0, 0:256], xT_sb[:, 0, 0:128],
                               vwt[:, 0, 0:256], start=True, stop=False)
                            mm(vp[:, 0, 256:512], xT_sb[:, 0, 0:128],
                               vwt[:, 0, 256:512], start=False, stop=False)
                            continue
                        mm(
                            vp[:, i, :],
                            xT_sb[:, ct, _s(i, 128)],
                            vwt[:, ci, :],
                            start=(ct == 0),
                            stop=False,
                        )
                    for j in range(NKV):
                        mm(
                            kps[j][:],
                            kwt[:, ci, _s(j, HD)],
                            xT_sb[:, ct, :],
                            start=(ct == 0),
                            stop=False,
                        )
            if nch == 3:
                # defer the tail xT tiles behind this group's weight DMA:
                # their transfers would otherwise compete with the critical
                # first pieces for SDMA bandwidth in the 8-15us window
                # (they are not consumed until ~30us)
                nc.gpsimd.tensor_copy(gate_sb[0:1, 0:1], vwt[0:1, 0, 0:1])
                for ct in range(10, CT):
                    nc.gpsimd.dma_start(xT_sb[:, ct, :], xT[:, ct, :])
                # rope tables and mask are not consumed until ~50us --
                # keep them out of the critical early window too
                nc.gpsimd.dma_start(cosT_sb[:], cosT)
                nc.gpsimd.dma_start(sinT_sb[:], sinT)
                nc.gpsimd.dma_start(maskT_sb[:], maskT)
            if nch in _fillers:
                # zero-contribution filler matmuls (all-zero stationary
                # accumulates 0 into the open vp group) keep the PE busy --
                # and the HAM clock gate warm -- while the next group's
                # weights stream in
                for _w in range(_fillers[nch]):
                    mm(vp[:, 0, 0:256], warm_sb[:], xT_sb[:, 0, 0:256],
                       start=False, stop=False)

        for h in range(3):
            wqh_dma(h, nc.sync)

        rope(kps[0][:], kT_sb[:, 0, :], "k0")
        rope(kps[1][:], kT_sb[:, 1, :], "k1")

        # v_sb copies on ACT (after the k-rope copies in its queue): AV
        # doesn't need v_sb until iteration 3, and the in-order DVE queue
        # must not block rope-k0 (which gates the first q-head's PSUM slot)
        for i in range(TT):
            nc.scalar.copy(v_sb[:, i, :], vp[:, i, :])

        # ---- phase B: per q head, software pipelined ----
        state = {}

        def stage_a(h):  # projection matmuls into psum
            qp = ps.tile([HD, T], F32, tag="qp", bufs=2, name=f"qp{h}")
            wqh = wqh_tiles.pop(h)
            for ct in range(CT):
                mm(
                    qp[:],
                    wqh[:, ct, :],
                    xT_sb[:, ct, :],
                    start=(ct == 0),
                    stop=(ct == CT - 1),
                )
            wqh_dma(h + 3)
            state[h] = {"qp": qp}

        def stage_b(h):  # rope (straight from psum) -> qT
            qT = work.tile([HD, T], MM_DT, tag="qT", bufs=2, name=f"qT{h}")
            rope(state[h]["qp"][:], qT[:], f"q{h}")
            state[h]["qT"] = qT

        def stage_c1(h):  # scoresT matmuls, exp, mask (per s-tile bank)
            j = h // REP
            sT = ps.tile([128, TT, T], F32, tag="big", bufs=1, name=f"sT{h}")
            qT = state[h]["qT"]
            for i in range(TT):
                lo = 128 * i if causal else 0
                mm(
                    sT[:, i, lo:],
                    kT_sb[:, j, _s(i, 128)],
                    qT[:, lo:],
                    start=True,
                    stop=True,
                )
            expm = work.tile([128, TT, T], MM_DT, tag="expm", bufs=3, name=f"expm{h}")
            for i in range(TT):
                lo = 128 * i if causal else 0
                nc.scalar.activation(
                    expm[:, i, lo:], sT[:, i, lo:], AF.Exp, scale=SCALE
                )
                if causal:
                    # only the diagonal 128x128 block is partially masked;
                    # t < lo is never read downstream, t >= lo+128 is fully
                    # visible
                    nc.vector.tensor_mul(
                        expm[:, i, lo : lo + 128],
                        expm[:, i, lo : lo + 128],
                        maskT_sb[:, i, lo : lo + 128],
                    )
                else:
                    nc.vector.tensor_mul(
                        expm[:, i, :], expm[:, i, :], maskT_sb[:, i, :]
                    )
            if causal:
                # range-wise fold of the 4 s-tiles on GpSimd (otherwise idle
                # in phase B), reading only each tile's valid t-range, so the
                # softmax normalizer needs one N=512 ones-matmul instead of
                # four (saves 768 PE cycles per head)
                folda = work.tile([128, T], MM_DT, tag="folda", bufs=2, name=f"fa{h}")
                foldb = work.tile([128, T // 2], MM_DT, tag="foldb", bufs=2, name=f"fb{h}")
                nc.vector.tensor_copy(folda[:, 0:128], expm[:, 0, 0:128])
                nc.vector.tensor_add(
                    folda[:, 128:T], expm[:, 0, 128:T], expm[:, 1, 128:T]
                )
                nc.vector.tensor_copy(foldb[:, 0:128], expm[:, 2, 256:384])
                nc.vector.tensor_add(
                    foldb[:, 128:256], expm[:, 2, 384:T], expm[:, 3, 384:T]
                )
                nc.vector.tensor_add(folda[:, 256:T], folda[:, 256:T], foldb[:])
                state[h]["fold"] = folda
            state[h]["expm"] = expm

        def stage_c2(h):  # AV + normalizer matmuls, reciprocal, scale into aout
            j = h // REP
            expm = state[h]["expm"]
            avp = ps.tile([HD, T], F32, tag="av", bufs=1, name=f"avp{h}")
            for i in range(TT):
                lo = 128 * i if causal else 0
                mm(
                    avp[:, lo:],
                    v_sb[:, i, _s(j, HD)],
                    expm[:, i, lo:],
                    start=(i == 0),
                    stop=(i == TT - 1),
                )
            lp = ps.tile([128, T], F32, tag="lsum", bufs=1, name=f"lp{h}")
            if causal:
                mm(lp[:], ones_sb[:], state[h]["fold"][:], start=True, stop=True)
            else:
                for i in range(TT):
                    mm(
                        lp[:, 0:],
                        ones_sb[:],
                        expm[:, i, :],
                        start=(i == 0),
                        stop=(i == TT - 1),
                    )
            recip = work.tile([HD, T], F32, tag="recip", name=f"recip{h}")
            nc.vector.reciprocal_approx_fast(recip[:], lp[:HD, :])
            nc.vector.tensor_mul(aout_slice(h), avp[:], recip[:])
            del state[h]

        # phase C weight prefetch for the first column group, issued early so
        # the drain-interleaved partial chains below have their wo chunks
        wots0 = []
        y_partials = {}

        def emit_wots0():
            for fg in range(4):
                wot = streams.tile(
                    [128, 4, 512], MM_DT, tag="wkv4", bufs=8, name=f"wot0_{fg}"
                )
                nc.sync.dma_start(wot[:], wo[:, 0, fg])
                wots0.append(wot)

        def start_y_partial(i, tag="qp", bufs=2):
            # partial eg0 y-tile accumulation over heads 0..11 (aout groups
            # 0-2, complete by c2(11)) — fills the PE while the last heads'
            # exp/AV/normalizer chains drain on ACT/DVE
            ypt = ps.tile([128, 512], F32, tag=tag, bufs=bufs, name=f"yp0_{i}")
            for ft in range(12):
                mm(
                    ypt[:],
                    aout_slice(ft, _s(i, 128)),
                    wots0[ft // 4][:, ft % 4, :],
                    start=(ft == 0),
                    stop=False,
                )
            y_partials[i] = ypt

        for it in range(NH + 3):
            if it < NH:
                stage_a(it)
            if it < 2:
                rope(kps[it + 2][:], kT_sb[:, it + 2, :], f"k{it + 2}")
            if 0 <= it - 3 < NH:
                stage_c2(it - 3)
            if it < NH:
                stage_b(it)
            if 0 <= it - 2 < NH:
                stage_c1(it - 2)
            if it == NH - 3:
                emit_wots0()
            if it == NH + 1:
                start_y_partial(0)
                start_y_partial(1)
            if it == NH + 2:
                # third partial in the "big" slot (free after the last exp
                # read): no head-15 dependency, so it covers the final
                # recip/aout-mul latency on the DVE
                start_y_partial(2, tag="big", bufs=1)

        # ---- phase C: output projection y = aout.T @ wo ----
        # per (eg, i) y-tile: run the full 16-step accumulation in one PSUM
        # bank, then copy+DMA immediately so the output drains continuously
        # and only one small tile trails the last matmul
        for eg in range(EG):
            if eg == 0:
                wots = wots0
            else:
                wots = []
                for fg in range(4):
                    wot = streams.tile(
                        [128, 4, 512], MM_DT, tag="wkv4", bufs=8, name=f"wot{eg}_{fg}"
                    )
                    nc.sync.dma_start(wot[:], wo[:, eg, fg])
                    wots.append(wot)
            for i in range(TT):
                last = eg == EG - 1 and i == TT - 1
                if not last:
                    if eg == 0 and i in y_partials:
                        ypt = y_partials[i]
                        ft0 = 12
                    else:
                        ypt = ps.tile(
                            [128, 512], F32, tag="qp", bufs=2, name=f"yp{eg}_{i}"
                        )
                        ft0 = 0
                    for ft in range(ft0, CT):
                        mm(
                            ypt[:],
                            aout_slice(ft, _s(i, 128)),
                            wots[ft // 4][:, ft % 4, :],
                            start=(ft == 0),
                            stop=(ft == CT - 1),
                        )
                    ysb_i = work.tile(
                        [128, 512], MM_DT, tag="ysb1", bufs=4, name=f"ysb{eg}_{i}"
                    )
                    if i % 2 == 0:
                        nc.scalar.copy(ysb_i[:], ypt[:])
                    else:
                        nc.vector.tensor_copy(ysb_i[:], ypt[:])
                    (nc.scalar if i % 2 == 0 else nc.sync).dma_start(
                        y[_s(i, 128), _s(eg, 512)], ysb_i[:]
                    )
                else:
                    # final tile as two 256-col chains in separate PSUM
                    # banks so only a 64KB copy+DMA trails the last matmul
                    for cc in range(2):
                        cs = slice(cc * 256, (cc + 1) * 256)
                        ypc = ps.tile(
                            [128, 256], F32, tag="qp", bufs=2, name=f"ypl{cc}"
                        )
                        for ft in range(CT):
                            mm(
                                ypc[:],
                                aout_slice(ft, _s(i, 128)),
                                wots[ft // 4][:, ft % 4, cs],
                                start=(ft == 0),
                                stop=(ft == CT - 1),
                            )
                        ysb_c = work.tile(
                            [128, 256], MM_DT, tag="ysbl", bufs=2, name=f"ysbl{cc}"
                        )
                        if cc % 2 == 0:
                            nc.scalar.copy(ysb_c[:], ypc[:])
                            nc.scalar.dma_start(
                                y[_s(i, 128), eg * 512 : eg * 512 + 256],
                                ysb_c[:],
                            )
                        else:
                            nc.vector.tensor_copy(ysb_c[:], ypc[:])
                            # final 64KB store split across both HWDGE rings
                            # so the two receipts overlap
                            nc.sync.dma_start(
                                y[_s(i, 128), eg * 512 + 256 : eg * 512 + 384],
                                ysb_c[:, 0:128],
                            )
                            nc.scalar.dma_start(
                                y[_s(i, 128), eg * 512 + 384 : eg * 512 + 512],
                                ysb_c[:, 128:256],
                            )


def build(causal=False):
    nc = bacc.Bacc(
        "TRN2",
        target_bir_lowering=False,
        debug=False,
        enable_asserts=False,
        num_devices=N_CORES,
    )
    # all inputs are pre-tiled on the host so every DMA is per-partition
    # contiguous (the partition index p is always the first axis)
    xT = nc.dram_tensor("xT", [128, CT, T], MM_DT, kind="ExternalInput").ap()
    wq = nc.dram_tensor("wq", [128, NH, CT, HD], MM_DT, kind="ExternalInput").ap()
    wk = nc.dram_tensor("wk", [128, CT, NKV * HD], MM_DT, kind="ExternalInput").ap()
    wv = nc.dram_tensor("wv", [128, CT, NKV * HD], MM_DT, kind="ExternalInput").ap()
    wo = nc.dram_tensor("wo", [128, EG, 4, 4, 512], MM_DT, kind="ExternalInput").ap()
    cosT = nc.dram_tensor("cosT", [HD, T], MM_DT, kind="ExternalInput").ap()
    sinT = nc.dram_tensor("sinT", [HD, T], MM_DT, kind="ExternalInput").ap()
    maskT = nc.dram_tensor("maskT", [128, TT, T], MM_DT, kind="ExternalInput").ap()
    y = nc.dram_tensor("y", [T, C], MM_DT, kind="ExternalOutput").ap()

    with tile.TileContext(nc) as tc:
        _emit(tc, xT, wq, wk, wv, wo, cosT, sinT, maskT, y, causal)
    nc.compile()
    return nc


_NC = {}


def _get_nc(causal):
    if causal not in _NC:
        _NC[causal] = build(causal)
    return _NC[causal]


def _is_causal(mask):
    return bool(np.array_equal(mask, np.tril(np.ones((T, T), dtype=bool))))


def host_tables():
    """cos/sin tables (transposed) and the signed rotate-half matrix."""
    inv = 1.0 / (10000.0 ** (np.arange(0, HD, 2, dtype=np.float32) / HD))
    t = np.arange(T, dtype=np.float32)
    freqs = np.outer(t, inv)  # [T, HD/2]
    emb = np.concatenate([freqs, freqs], axis=-1)  # [T, HD]
    cosT = np.ascontiguousarray(np.cos(emb).T, dtype=np.float32)
    sinT = np.ascontiguousarray(np.sin(emb).T, dtype=np.float32)
    # rotate-half signs baked in: rows d<64 multiply the shifted-down half
    # with a minus sign (q'[d] = q[d]cos - q[d+64]sin for d<64)
    sinT[: HD // 2] *= -1.0
    return cosT, sinT


def make_in_maps(inputs):
    x = np.asarray(inputs["x"], dtype=np.float32)
    mask = np.asarray(inputs["mask"]).reshape(T, T)
    cosT, sinT = host_tables()
    wq = np.asarray(inputs["wq"]).astype(MM_NP)
    wk = np.asarray(inputs["wk"]).astype(MM_NP)
    wv = np.asarray(inputs["wv"]).astype(MM_NP)
    wo = np.asarray(inputs["wo"]).astype(MM_NP)
    # pre-tile everything partition-major so each DMA reads per-partition
    # contiguous ranges: layout[p, ...] with source row = tile*128 + p
    shared = {
        "wq": np.ascontiguousarray(
            wq.reshape(CT, 128, NH, HD).transpose(1, 2, 0, 3)
        ),
        "wk": np.ascontiguousarray(
            wk.reshape(CT, 128, NKV * HD).transpose(1, 0, 2)
        ),
        "wv": np.ascontiguousarray(
            wv.reshape(CT, 128, NKV * HD).transpose(1, 0, 2)
        ),
        "wo": np.ascontiguousarray(
            wo.reshape(4, 4, 128, EG, 512).transpose(2, 3, 0, 1, 4)
        ),
        "cosT": cosT.astype(MM_NP),
        "sinT": sinT.astype(MM_NP),
        "maskT": np.ascontiguousarray(
            mask.T.astype(MM_NP).reshape(TT, 128, T).transpose(1, 0, 2)
        ),
    }
    return [
        {
            "xT": np.ascontiguousarray(
                x[b].T.astype(MM_NP).reshape(CT, 128, T).transpose(1, 0, 2)
            ),
            **shared,
        }
        for b in range(N_CORES)
    ]


def run(inputs, **kw):
    mask = np.asarray(inputs["mask"]).reshape(T, T)
    nc = _get_nc(_is_causal(mask))
    in_maps = make_in_maps(inputs)
    res = run_bass_kernel_spmd(nc, in_maps, core_ids=list(range(N_CORES)), **kw)
    out = np.stack([r["y"] for r in res.results], axis=0).astype(np.float32)
    return out, res


def kernel(**inputs) -> np.ndarray:
    out, _ = run(inputs)
    return out

